# revision 2
# baseline (speedup 1.0000x reference)
"""PositionalSparseLinear on 8 Trainium2 NeuronCores.

out[b, o] = sum_k x[b, connections[o, k]] * weights[o, k]
B=1024, In=8192, O=8192, K=32.

Strategy: out-feature sharding (1024 outputs per core). On the host we build a
sparse scatter matrix S[f, o] = sum_k weights[o,k] * [connections[o,k] == f]
(collisions summed), cast to fp16. On-device each core computes
out_c = x @ S_c as 64 chunk-matmuls (contraction dim = 128 features) that
accumulate in PSUM (fp32), with the xT operand resident in SBUF (fp16) and the
S blocks streamed from DRAM, double buffered. Raw Bass (explicit semaphores).
"""

import sys

sys.path.insert(0, "/opt/trn_rl_repo")

import numpy as np

import concourse.bass as bass
import concourse.mybir as mybir
from concourse.bass_utils import run_bass_kernel_spmd

B = 1024          # batch
IN = 8192         # in features
O = 8192          # out features
K = 32            # connections per output
NCORES = 8
OC = O // NCORES  # outputs per core (1024)
NT = OC // 128    # o-tiles per core (8)
NCH = IN // 128   # feature chunks (64)
NBH = B // 512    # b-halves (2)

F16 = mybir.dt.float16
F32 = mybir.dt.float32

_cached = None


def _build_program():
    nc = bass.Bass()
    xts_in = nc.declare_dram_parameter("xts", [128, NCH, B], F16, isOutput=False)
    s_in = nc.declare_dram_parameter("s", [NT, 128, NCH, 128], F16, isOutput=False)
    y_out = nc.declare_dram_parameter("y", [NT, 128, B], F32, isOutput=True)

    with (
        nc.sbuf_tensor("xts_sb", [128, NCH, B], F16) as xts,
        nc.sbuf_tensor("sbuf_s", [128, 2, NCH, 128], F16) as s_sb,
        nc.sbuf_tensor("out_sb", [128, 2, B], F32) as out_sb,
        nc.psum_tensor("ps0", [128, 512], F32) as ps0,
        nc.psum_tensor("ps1", [128, 512], F32) as ps1,
        nc.psum_tensor("ps2", [128, 512], F32) as ps2,
        nc.psum_tensor("ps3", [128, 512], F32) as ps3,
        nc.Block() as block,
        nc.semaphore("x_sem") as x_sem,        # xts load done
        nc.semaphore("s_sem") as s_sem,        # S[t] loads done (16 per t)
        nc.semaphore("pe_sem") as pe_sem,      # PE finished o-tile t (1 per t)
        nc.semaphore("v_sem") as v_sem,        # DVE copied o-tile t (1 per t)
        nc.semaphore("yd_sem") as yd_sem,      # y DMA done (16 per t)
    ):
        psum = [(ps0, ps1), (ps2, ps3)]  # [t % 2][b half]

        @block.sync
        def _(sync: bass.BassEngine):
            # S[0], S[1] first (small) so PE can start as soon as xts lands.
            sync.dma_start(out=s_sb[:, 0], in_=s_in[0]).then_inc(s_sem, 16)
            sync.dma_start(out=s_sb[:, 1], in_=s_in[1]).then_inc(s_sem, 16)
            sync.dma_start(out=xts[:], in_=xts_in[:]).then_inc(x_sem, 16)
            for t in range(2, NT):
                # buffer t%2 is free once PE finished o-tile t-2
                sync.wait_ge(pe_sem, t - 1)
                sync.dma_start(out=s_sb[:, t % 2], in_=s_in[t]).then_inc(s_sem, 16)

        @block.tensor
        def _(pe: bass.BassEngine):
            pe.wait_ge(x_sem, 16)
            for t in range(NT):
                pe.wait_ge(s_sem, 16 * (t + 1))
                if t >= 2:
                    # psum bank pair t%2 free once DVE copied o-tile t-2
                    pe.wait_ge(v_sem, t - 1)
                for ic in range(NCH):
                    for bh in range(NBH):
                        mm = pe.matmul(
                            out=psum[t % 2][bh][:],
                            lhsT=s_sb[:, t % 2, ic, :],
                            rhs=xts[:, ic, bh * 512:(bh + 1) * 512],
                            start=(ic == 0),
                            stop=(ic == NCH - 1),
                        )
                        if ic == NCH - 1 and bh == NBH - 1:
                            mm.then_inc(pe_sem, 1)

        @block.vector
        def _(vector: bass.BassEngine):
            for t in range(NT):
                vector.wait_ge(pe_sem, t + 1)
                if t >= 2:
                    # out_sb buffer t%2 free once its y DMA completed
                    vector.wait_ge(yd_sem, 16 * (t - 1))
                vector.tensor_copy(out=out_sb[:, t % 2, 0:512], in_=psum[t % 2][0][:])
                vector.tensor_copy(
                    out=out_sb[:, t % 2, 512:1024], in_=psum[t % 2][1][:]
                ).then_inc(v_sem, 1)

        @block.scalar
        def _(scalar: bass.BassEngine):
            for t in range(NT):
                scalar.wait_ge(v_sem, t + 1)
                scalar.dma_start(out=y_out[t], in_=out_sb[:, t % 2]).then_inc(yd_sem, 16)
            scalar.wait_ge(yd_sem, 16 * NT)

    return nc


def _prep_inputs(x, connections, weights):
    # xT in [partition=f%128, chunk=f//128, b] layout, fp16
    xT = np.ascontiguousarray(x.T.astype(np.float16))          # [IN, B]
    xts = np.ascontiguousarray(
        xT.reshape(NCH, 128, B).transpose(1, 0, 2)
    )                                                          # [128, NCH, B]

    # scatter matrix per core: S_all[c, t, ic, p, j]
    o = np.arange(O).repeat(K)                                 # [O*K]
    f = connections.reshape(-1).astype(np.int64)               # [O*K]
    w = weights.reshape(-1).astype(np.float32)
    c = o // OC
    t = (o % OC) // 128
    j = o % 128
    ic = f // 128
    p = f % 128
    S = np.zeros((NCORES, NT, NCH, 128, 128), dtype=np.float32)
    np.add.at(S, (c, t, ic, p, j), w)
    S = S.astype(np.float16)
    # reorder to [c, t, p, ic, j] so each per-t slice DMA is contiguous
    S = np.ascontiguousarray(S.transpose(0, 1, 3, 2, 4))
    return xts, S


def kernel(x, connections, weights):
    global _cached
    if _cached is None:
        _cached = _build_program()
    nc = _cached
    xts, S = _prep_inputs(np.asarray(x), np.asarray(connections), np.asarray(weights))
    in_maps = [{"xts": xts, "s": S[cid]} for cid in range(NCORES)]
    res = run_bass_kernel_spmd(nc, in_maps, core_ids=list(range(NCORES)))
    out = np.empty((B, O), dtype=np.float32)
    for cid in range(NCORES):
        y = res.results[cid]["y"]                              # [NT, 128, B]
        out[:, cid * OC:(cid + 1) * OC] = (
            y.reshape(OC, B).T                                 # [B, OC]
        )
    return out
